# revision 1
# baseline (speedup 1.0000x reference)
"""MoE top-2 routing kernel for 8 Trainium2 NeuronCores.

Strategy (expert-parallel, per spec sharding hint):
  - Host computes the (cheap) gate: softmax -> top-2 -> renormalized scores.
  - Tokens are gathered per expert on the host; core e runs a dense
    gelu-MLP (fc1 -> gelu -> fc2) for expert e over its gathered tokens.
  - Device matmuls run in float32r (1 cycle/row on the PE when the moving
    dim >= 256, i.e. full speed) streaming weights from DRAM; the host
    applies the combine scores + b2 and scatter-adds the results back.

Per-core device work (C ~= 4608 padded tokens):
  fc1: h = gelu(x @ w1 + b1)   [C,1024] @ [1024,4096]
  fc2: y = h @ w2              [C,4096] @ [4096,1024]
  PE floor ~= C * 512 cycles / 2.4GHz ~= 1.0 ms; DMA ~= 320MB overlapped.
"""

import sys

sys.path.insert(0, "/opt/trn_rl_repo")

from contextlib import ExitStack

import numpy as np

from concourse import bacc, mybir, tile
from concourse.bass_utils import run_bass_kernel_spmd

E, H, I = 8, 1024, 4096
TOP_K = 2
N_CORES = 8

TT = 512  # token tile (two PSUM subtiles of 256)
SUB = TT // 2

F32 = mybir.dt.float32
F32R = mybir.dt.float32r


def _build_nc(C: int, act_func=None, repeat=1):
    """One SPMD program: dense expert MLP over C gathered tokens."""
    nc = bacc.Bacc(
        "TRN2", target_bir_lowering=False, debug=False, num_devices=N_CORES
    )
    xT = nc.dram_tensor("xT", [8, 128, C], F32R, kind="ExternalInput").ap()
    w1p = nc.dram_tensor("w1p", [8, 128, 4, 1024], F32R, kind="ExternalInput").ap()
    b1p = nc.dram_tensor("b1p", [128, 32], F32, kind="ExternalInput").ap()
    w2p = nc.dram_tensor("w2p", [8, 128, 4096], F32R, kind="ExternalInput").ap()
    yT = nc.dram_tensor("yT", [8, 128, C], F32, kind="ExternalOutput").ap()

    n_tiles = C // TT
    gelu = act_func if act_func is not None else mybir.ActivationFunctionType.Gelu

    with tile.TileContext(nc) as tc, ExitStack() as ctx:
        b1pool = ctx.enter_context(tc.tile_pool(name="b1", bufs=1))
        xpool = ctx.enter_context(tc.tile_pool(name="x", bufs=2))
        hpool = ctx.enter_context(tc.tile_pool(name="h", bufs=1))
        w1pool = ctx.enter_context(tc.tile_pool(name="w1", bufs=2))
        w2pool = ctx.enter_context(tc.tile_pool(name="w2", bufs=2))
        ypool = ctx.enter_context(tc.tile_pool(name="y", bufs=3))
        ps1 = ctx.enter_context(tc.tile_pool(name="ps1", bufs=3, space="PSUM"))
        ps2 = ctx.enter_context(tc.tile_pool(name="ps2", bufs=3, space="PSUM"))

        b1t = b1pool.tile([128, 32], F32)
        nc.sync.dma_start(out=b1t[:], in_=b1p)

        rep_ctx = tc.For_i(0, repeat, 1) if repeat > 1 else None
        if rep_ctx is not None:
            ctx.enter_context(rep_ctx)

        for t in range(n_tiles):
            t0 = t * TT
            xt = xpool.tile([128, 8, TT], F32R)
            for kc in range(8):
                nc.sync.dma_start(out=xt[:, kc, :], in_=xT[kc, :, t0 : t0 + TT])

            ht = hpool.tile([128, 32, TT], F32R)

            # fc1 + gelu: 32 output chunks of 128, contraction over 8 k-chunks
            for ms in range(8):  # w1 slabs of 4 m-chunks (2MB DMA each)
                w1t = w1pool.tile([128, 4, 1024], F32R)
                nc.gpsimd.dma_start(out=w1t[:], in_=w1p[ms])
                for j in range(4):
                    mc = ms * 4 + j
                    for sub in range(2):
                        ps = ps1.tile([128, SUB], F32)
                        for kc in range(8):
                            nc.tensor.matmul(
                                ps[:],
                                lhsT=w1t[:, j, kc * 128 : (kc + 1) * 128],
                                rhs=xt[:, kc, sub * SUB : (sub + 1) * SUB],
                                start=(kc == 0),
                                stop=(kc == 7),
                            )
                        nc.scalar.activation(
                            out=ht[:, mc, sub * SUB : (sub + 1) * SUB],
                            in_=ps[:],
                            func=gelu,
                            bias=b1t[:, mc : mc + 1],
                            scale=1.0,
                        )

            # fc2: 8 output chunks of 128, contraction over 32 i-chunks
            for m2c in range(8):
                w2t = w2pool.tile([128, 4096], F32R)
                nc.gpsimd.dma_start(out=w2t[:], in_=w2p[m2c])
                yt = ypool.tile([128, TT], F32)
                for sub in range(2):
                    ps = ps2.tile([128, SUB], F32)
                    for ic in range(32):
                        nc.tensor.matmul(
                            ps[:],
                            lhsT=w2t[:, ic * 128 : (ic + 1) * 128],
                            rhs=ht[:, ic, sub * SUB : (sub + 1) * SUB],
                            start=(ic == 0),
                            stop=(ic == 31),
                        )
                    nc.vector.tensor_copy(
                        out=yt[:, sub * SUB : (sub + 1) * SUB], in_=ps[:]
                    )
                nc.sync.dma_start(out=yT[m2c, :, t0 : t0 + TT], in_=yt[:])

    nc.compile()
    return nc


def _route(x_flat, gate_w, gate_b, alpha):
    """Host gate: returns per-expert (row_indices, combine_scores)."""
    logits = x_flat @ gate_w + gate_b
    m = logits.max(axis=-1, keepdims=True)
    p = np.exp(logits - m)
    p /= p.sum(axis=-1, keepdims=True)
    idx = np.argpartition(p, E - TOP_K, axis=-1)[:, -TOP_K:]  # top-2 (unordered)
    vals = np.take_along_axis(p, idx, axis=-1)
    sc = vals / vals.sum(axis=-1, keepdims=True)
    sc = sc * alpha[idx]
    routes = []
    for e in range(E):
        mask = idx == e  # at most one True per row (top-k distinct)
        rows = np.nonzero(mask.any(axis=1))[0]
        scores = sc[mask]  # row-major order matches `rows`
        routes.append((rows, scores.astype(np.float32)))
    return routes


def prepare(hidden_states, gate_w, gate_b, w1, b1, w2, b2, alpha):
    """Host routing + input prep. Returns (nc, in_maps, state)."""
    x = np.asarray(hidden_states, dtype=np.float32)
    gate_w = np.asarray(gate_w, dtype=np.float32)
    gate_b = np.asarray(gate_b, dtype=np.float32)
    w1 = np.asarray(w1, dtype=np.float32)
    b1 = np.asarray(b1, dtype=np.float32)
    w2 = np.asarray(w2, dtype=np.float32)
    b2 = np.asarray(b2, dtype=np.float32)
    alpha = np.asarray(alpha, dtype=np.float32)

    B, S, Hd = x.shape
    T = B * S
    xf = x.reshape(T, Hd)

    routes = _route(xf, gate_w, gate_b, alpha)
    max_cnt = max(len(r) for r, _ in routes)
    C = max(TT, ((max_cnt + TT - 1) // TT) * TT)

    nc = _build_nc(C)

    in_maps = []
    for e in range(E):
        rows, _ = routes[e]
        xTe = np.zeros((8, 128, C), dtype=np.float32)
        if len(rows):
            xTe[:, :, : len(rows)] = (
                xf[rows].T.reshape(8, 128, len(rows))
            )
        w1pe = np.ascontiguousarray(
            w1[e].reshape(8, 128, 32, 128).transpose(2, 1, 0, 3).reshape(32, 128, 1024)
            .reshape(8, 4, 128, 1024).transpose(0, 2, 1, 3)
        )
        b1pe = np.ascontiguousarray(b1[e].reshape(32, 128).T)
        w2pe = np.ascontiguousarray(
            w2[e].reshape(32, 128, 8, 128).transpose(2, 1, 0, 3).reshape(8, 128, 4096)
        )
        in_maps.append({"xT": xTe, "w1p": w1pe, "b1p": b1pe, "w2p": w2pe})

    state = dict(routes=routes, C=C, b2=b2, B=B, S=S, Hd=Hd, T=T)
    return nc, in_maps, state


def finalize(results, state):
    routes, C, b2 = state["routes"], state["C"], state["b2"]
    T, Hd = state["T"], state["Hd"]
    out = np.zeros((T, Hd), dtype=np.float32)
    for e in range(E):
        rows, scores = routes[e]
        if not len(rows):
            continue
        yTe = results[e]["yT"].reshape(1024, C)
        ye = yTe[:, : len(rows)].T  # [cnt, 1024]
        out[rows] += scores[:, None] * (ye + b2[e])
    return out.reshape(state["B"], state["S"], Hd)


def kernel(hidden_states, gate_w, gate_b, w1, b1, w2, b2, alpha):
    nc, in_maps, state = prepare(
        hidden_states, gate_w, gate_b, w1, b1, w2, b2, alpha
    )
    res = run_bass_kernel_spmd(nc, in_maps, list(range(N_CORES)))
    return finalize(res.results, state)

